# revision 1
# baseline (speedup 1.0000x reference)
"""Causal self-attention with RoPE (B=2, T=1024, C=2048, H=16) on 8 TRN2
NeuronCores, head-parallel tensor sharding (2 heads per core).

Per-core Bass/Tile kernel:
  0. x^T arrives channel-sharded (256 rows per core) and is AllGathered
     on-chip into internal DRAM (kernel I/O staging is far slower than HBM,
     so every replicated byte is gathered on-chip instead). RoPE tables ride
     the same trick.
  1. QKV projections with fp32r matmuls (weights column-sharded; contraction
     dim on partitions).
  2. RoPE applied to q^T/k^T via an SBUF partition-shift DMA + DVE FMA.
  3. Causal attention in [tk, tq] score layout: exp on ScalarE straight out of
     PSUM, softmax denominator via an all-ones-lhsT matmul (broadcast across
     partitions for free), unnormalized y accumulated in PSUM, one reciprocal
     + multiply at the end. Fully-masked tiles are skipped.
  4. AllGather of the per-core y^T shard (one per batch, overlapped).
  5. Output projection of this core's 256 output columns from gathered y^T.
Host reassembles: concat core outputs along the channel dim.
"""
import numpy as np

import concourse.bass as bass
import concourse.mybir as mybir
import concourse.tile as tile
from concourse import bacc
from concourse.bass_utils import run_bass_kernel_spmd

F32 = mybir.dt.float32
F32R = mybir.dt.float32r
BF16 = mybir.dt.bfloat16
STAGE_BF16 = True          # stage x/weights as bf16 (halves slow kernel-I/O)
SDT = BF16 if STAGE_BF16 else F32R

B, T, C = 2, 1024, 2048
H = 16
D = C // H            # 128
BT = B * T            # 2048
NCORES = 8
HL = H // NCORES      # heads per core = 2
CL = HL * D           # local channels = 256
ATT_SCALE = 1.0 / float(np.sqrt(D))
ROPE_BASE = 10000.0
NEG = -1.0e30

CT = C // 128         # 16 contraction tiles
TB = BT // 512        # 4 token blocks of 512
CSH = 2 * D // NCORES  # rows of the cos/sin shard per core = 32
RG = [list(range(NCORES))]


def _round_f32r(x: np.ndarray) -> np.ndarray:
    """Round-to-nearest-even to the 10-bit-mantissa fp32r grid."""
    u = np.ascontiguousarray(x, dtype=np.float32).view(np.uint32)
    low = np.uint32(0x1FFF)
    half = np.uint32(0x0FFF)
    r = (u + half + ((u >> np.uint32(13)) & np.uint32(1))) & ~low
    return r.view(np.float32)


def _rope_tables():
    inv_freq = 1.0 / (ROPE_BASE ** (np.arange(0, D, 2, dtype=np.float64) / D))
    t = np.arange(T, dtype=np.float64)
    freqs = np.outer(t, inv_freq)                       # [T, D/2]
    emb = np.concatenate([freqs, freqs], axis=-1)        # [T, D]
    cos = np.cos(emb).astype(np.float32)                 # [T, D]
    sin = np.sin(emb).astype(np.float32)
    cosT = np.ascontiguousarray(cos.T)                   # [D, T]
    sinT = np.ascontiguousarray(sin.T)
    sgn_sinT = sinT.copy()
    sgn_sinT[: D // 2] *= -1.0                           # rotate_half sign
    cosT_full = np.ascontiguousarray(np.tile(cosT, (1, B)))       # [D, BT]
    sgn_sinT_full = np.ascontiguousarray(np.tile(sgn_sinT, (1, B)))
    return cosT_full, sgn_sinT_full


def _build(use_collective=True):
    nc = bacc.Bacc("TRN2", target_bir_lowering=False, debug=False,
                   num_devices=NCORES)

    # channel-sharded x^T: rows [m*256, (m+1)*256) of the [C, BT] matrix
    xts_d = nc.dram_tensor("xts", [CL, BT], SDT, kind="ExternalInput").ap()
    # weights arrive pre-laid-out as [128, CT*CL]: partition p holds
    # WT[ct*128+p, o] at free offset ct*CL+o -> contiguous 8-16KB DRAM rows
    wqT_d = nc.dram_tensor("wqT", [128, CT * CL], SDT, kind="ExternalInput").ap()
    wkT_d = nc.dram_tensor("wkT", [128, CT * CL], SDT, kind="ExternalInput").ap()
    wvT_d = nc.dram_tensor("wvT", [128, CT * CL], SDT, kind="ExternalInput").ap()
    woT_d = nc.dram_tensor("woT", [128, CT * CL], SDT, kind="ExternalInput").ap()
    # cos/sin shard: rows [m*16,(m+1)*16) of cosT stacked on same of sgn_sinT
    css_d = nc.dram_tensor("css", [CSH, BT], F32, kind="ExternalInput").ap()
    out_d = nc.dram_tensor("out", [128, CT * CL], F32,
                           kind="ExternalOutput").ap()

    # internal DRAM (fast HBM): collective bounce buffers
    xtsb_d = nc.dram_tensor("xtsb", [CL, BT], SDT)
    xtg_d = nc.dram_tensor("xtg", [C, BT], SDT, addr_space="Shared")
    cssb_d = nc.dram_tensor("cssb", [CSH, BT], F32)
    csg_d = nc.dram_tensor("csg", [NCORES * CSH, BT], F32, addr_space="Shared")
    yin_d = [nc.dram_tensor(f"yin{b}", [CL, T], F32R) for b in range(B)]
    yout_d = [nc.dram_tensor(f"yout{b}", [C, T], F32R, addr_space="Shared")
              for b in range(B)]

    with tile.TileContext(nc) as tc:
        with (
            tc.tile_pool(name="wpool", bufs=1) as wpool,
            tc.tile_pool(name="const", bufs=1) as cpool,
            tc.tile_pool(name="qkv", bufs=1) as qkvpool,
            tc.tile_pool(name="xs", bufs=6) as xspool,
            tc.tile_pool(name="rope", bufs=2) as ropepool,
            tc.tile_pool(name="att", bufs=3) as attpool,
            tc.tile_pool(name="wo", bufs=4) as wopool,
        ):
            # ---- stage shards into internal DRAM and AllGather ----
            nc.sync.dma_start(out=xtsb_d.ap(), in_=xts_d)
            nc.scalar.dma_start(out=cssb_d.ap(), in_=css_d)
            if use_collective:
                nc.gpsimd.collective_compute(
                    "AllGather", mybir.AluOpType.bypass, replica_groups=RG,
                    ins=[xtsb_d.ap()], outs=[xtg_d.ap()])
                nc.gpsimd.collective_compute(
                    "AllGather", mybir.AluOpType.bypass, replica_groups=RG,
                    ins=[cssb_d.ap()], outs=[csg_d.ap()])
            else:
                nc.gpsimd.dma_start(out=xtg_d.ap()[0:CL, :], in_=xtsb_d.ap())
                nc.gpsimd.dma_start(out=csg_d.ap()[0:CSH, :], in_=cssb_d.ap())

            # ---- weights (staged inputs, overlap the gather) ----
            # q/k/v weights are used directly at the staging dtype; the Wo
            # weight is upcast once to f32r (the Wo matmul's other operand
            # is the f32r gathered y^T)
            wq_sb = wpool.tile([128, CT * CL], SDT, tag="wq")
            wk_sb = wpool.tile([128, CT * CL], SDT, tag="wk")
            wv_sb = wpool.tile([128, CT * CL], SDT, tag="wv")
            wo_sb = wpool.tile([128, CT * CL], F32R, tag="wo")
            if STAGE_BF16:
                wo_ld = wpool.tile([128, CT * CL], BF16, tag="wold")
            else:
                wo_ld = wo_sb
            for w_sb, w_d, eng in ((wq_sb, wqT_d, nc.sync),
                                   (wk_sb, wkT_d, nc.scalar),
                                   (wv_sb, wvT_d, nc.gpsimd),
                                   (wo_ld, woT_d, nc.gpsimd)):
                eng.dma_start(out=w_sb[:], in_=w_d)
            if STAGE_BF16:
                nc.vector.tensor_copy(wo_sb[:], wo_ld[:])

            # gathered table is [cosT(128 rows); sgn_sinT(128 rows)] because
            # rank m's shard is rows [32m, 32m+32) of that stack
            cos_sb = cpool.tile([D, BT], F32, tag="cos")
            sin_sb = cpool.tile([D, BT], F32, tag="sin")
            nc.scalar.dma_start(out=cos_sb[:], in_=csg_d.ap()[0:D, :])
            nc.sync.dma_start(out=sin_sb[:], in_=csg_d.ap()[D:2 * D, :])

            ones_f = cpool.tile([128, 128], F32, tag="onesf")
            nc.gpsimd.memset(ones_f[:], 1.0)
            ones_sb = cpool.tile([128, 128], F32R, tag="ones")
            nc.vector.tensor_copy(ones_sb[:], ones_f[:])

            # additive causal mask for diagonal 128x128 blocks:
            # rows=tk, cols=tq; keep (0.0) where tk <= tq else NEG
            mask_sb = cpool.tile([128, 128], F32, tag="mask")
            nc.gpsimd.memset(mask_sb[:], 0.0)
            nc.gpsimd.affine_select(
                out=mask_sb[:], in_=mask_sb[:],
                compare_op=mybir.AluOpType.is_ge,
                fill=NEG, base=0,
                pattern=[[1, 128]], channel_multiplier=-1,
            )

            # persistent qkv/y activations
            qT = [qkvpool.tile([D, BT], F32R, tag=f"qT{h}", name=f"qT{h}")
                  for h in range(HL)]
            kT = [qkvpool.tile([D, BT], F32R, tag=f"kT{h}", name=f"kT{h}")
                  for h in range(HL)]
            v_sb = qkvpool.tile([128, (BT // 128) * CL], F32R, tag="v")
            yT = [qkvpool.tile([D, BT], F32R, tag=f"yT{h}", name=f"yT{h}")
                  for h in range(HL)]

            # ---- phase 1: QKV projections + rope ----
            with tc.tile_pool(name="psqkv", bufs=1, space="PSUM") as psq:
                for tb in range(TB):
                    tcol = tb * 512
                    ps_q = [psq.tile([128, 512], F32, tag=f"pq{h}", name=f"pq{h}")
                            for h in range(HL)]
                    ps_k = [psq.tile([128, 512], F32, tag=f"pk{h}", name=f"pk{h}")
                            for h in range(HL)]
                    ps_v = [psq.tile([128, CL], F32, tag=f"pv{i}", name=f"pv{i}")
                            for i in range(4)]
                    for ct in range(CT):
                        xs = xspool.tile([128, 512], SDT, tag="xs")
                        eng = nc.sync if ct % 2 == 0 else nc.scalar
                        eng.dma_start(
                            out=xs[:],
                            in_=xtg_d.ap()[ct * 128:(ct + 1) * 128,
                                           tcol:tcol + 512],
                        )
                        st, sp = ct == 0, ct == CT - 1
                        for h in range(HL):
                            nc.tensor.matmul(
                                ps_q[h][:],
                                wq_sb[:, ct * CL + h * D: ct * CL + (h + 1) * D],
                                xs[:], start=st, stop=sp)
                            nc.tensor.matmul(
                                ps_k[h][:],
                                wk_sb[:, ct * CL + h * D: ct * CL + (h + 1) * D],
                                xs[:], start=st, stop=sp)
                        for i in range(4):
                            nc.tensor.matmul(
                                ps_v[i][:],
                                xs[:, i * 128:(i + 1) * 128],
                                wv_sb[:, ct * CL:(ct + 1) * CL],
                                start=st, stop=sp)
                    # rope on q, k; plain copy for v
                    for h in range(HL):
                        for name, ps, dst in (("q", ps_q[h], qT[h]),
                                              ("k", ps_k[h], kT[h])):
                            tmp = ropepool.tile([128, 512], F32, tag="rtmp")
                            nc.vector.tensor_copy(tmp[:], ps[:])
                            rot = ropepool.tile([128, 512], F32, tag="rrot")
                            nc.gpsimd.dma_start(out=rot[0:64, :],
                                                in_=tmp[64:128, :])
                            nc.gpsimd.dma_start(out=rot[64:128, :],
                                                in_=tmp[0:64, :])
                            t1 = ropepool.tile([128, 512], F32, tag="rt1")
                            nc.vector.tensor_mul(
                                t1[:], ps[:], cos_sb[:, tcol:tcol + 512])
                            t2 = ropepool.tile([128, 512], F32, tag="rt2")
                            nc.vector.tensor_mul(
                                t2[:], rot[:], sin_sb[:, tcol:tcol + 512])
                            nc.vector.tensor_add(
                                dst[:, tcol:tcol + 512], t1[:], t2[:])
                    for i in range(4):
                        gt = tb * 4 + i
                        nc.vector.tensor_copy(
                            v_sb[:, gt * CL:(gt + 1) * CL], ps_v[i][:])

            # ---- phase 2: attention + AllGather per batch ----
            with tc.tile_pool(name="psatt", bufs=1, space="PSUM") as psa:
                for b in range(B):
                    bcol = b * T
                    for h in range(HL):
                        for jj in range(2):
                            qcol = bcol + jj * 512
                            njt = 4 * jj + 4
                            ps_y = psa.tile([128, 512], F32, tag="y",
                                            bufs=2)
                            ps_l = psa.tile([128, 512], F32, tag="l",
                                            bufs=2)
                            for j in range(njt):
                                c0 = max(0, j * 128 - jj * 512)
                                ps_s = psa.tile([128, 512], F32, tag="s",
                                                bufs=3)
                                nc.tensor.matmul(
                                    ps_s[:, c0:512],
                                    kT[h][:, bcol + j * 128: bcol + (j + 1) * 128],
                                    qT[h][:, qcol + c0: qcol + 512],
                                    start=True, stop=True)
                                diag0 = j * 128 - jj * 512
                                if 0 <= diag0 < 512:
                                    nc.vector.tensor_add(
                                        ps_s[:, diag0:diag0 + 128],
                                        ps_s[:, diag0:diag0 + 128],
                                        mask_sb[:])
                                p = attpool.tile([128, 512], F32R, tag="p")
                                nc.scalar.activation(
                                    p[:, c0:512], ps_s[:, c0:512],
                                    mybir.ActivationFunctionType.Exp,
                                    scale=ATT_SCALE)
                                st, sp = j == 0, j == njt - 1
                                nc.tensor.matmul(
                                    ps_l[:, c0:512], ones_sb[:],
                                    p[:, c0:512], start=st, stop=sp)
                                gt = (bcol // 128) + j
                                nc.tensor.matmul(
                                    ps_y[:, c0:512],
                                    v_sb[:, gt * CL + h * D: gt * CL + (h + 1) * D],
                                    p[:, c0:512], start=st, stop=sp)
                            rec = attpool.tile([128, 512], F32, tag="rec")
                            nc.vector.reciprocal(rec[:], ps_l[:])
                            nc.vector.tensor_mul(
                                yT[h][:, qcol:qcol + 512], ps_y[:], rec[:])
                    # ship this batch's yT shard and AllGather it
                    for h in range(HL):
                        nc.sync.dma_start(
                            out=yin_d[b].ap()[h * D:(h + 1) * D, :],
                            in_=yT[h][:, bcol:bcol + T])
                    if use_collective:
                        nc.gpsimd.collective_compute(
                            "AllGather", mybir.AluOpType.bypass,
                            replica_groups=RG,
                            ins=[yin_d[b].ap()],
                            outs=[yout_d[b].ap()],
                        )
                    else:
                        nc.gpsimd.dma_start(out=yout_d[b].ap()[0:CL, :],
                                            in_=yin_d[b].ap())

            # ---- phase 3: output projection (this core's 256 columns) ----
            ob_big = qkvpool.tile([128, CT * CL], F32, tag="obig")
            with tc.tile_pool(name="pso", bufs=1, space="PSUM") as pso:
                for b in range(B):
                    for half in range(2):
                        bh = b * 2 + half
                        hcol = half * 512
                        ps_o = [pso.tile([128, CL], F32, tag=f"po{i}", name=f"po{i}")
                                for i in range(4)]
                        for ct in range(CT):
                            yg = wopool.tile([128, 512], F32R, tag="yg")
                            eng = nc.sync if ct % 2 == 0 else nc.scalar
                            eng.dma_start(
                                out=yg[:],
                                in_=yout_d[b].ap()[ct * 128:(ct + 1) * 128,
                                                   hcol:hcol + 512])
                            st, sp = ct == 0, ct == CT - 1
                            for i in range(4):
                                nc.tensor.matmul(
                                    ps_o[i][:],
                                    yg[:, i * 128:(i + 1) * 128],
                                    wo_sb[:, ct * CL:(ct + 1) * CL],
                                    start=st, stop=sp)
                        for i in range(4):
                            nc.vector.tensor_copy(
                                ob_big[:, bh * 1024 + i * CL:
                                       bh * 1024 + (i + 1) * CL],
                                ps_o[i][:])
                        nc.gpsimd.dma_start(
                            out=out_d[:, bh * 1024:(bh + 1) * 1024],
                            in_=ob_big[:, bh * 1024:(bh + 1) * 1024])

    nc.compile()
    return nc


_NC_CACHE = None


def _get_nc():
    global _NC_CACHE
    if _NC_CACHE is None:
        _NC_CACHE = _build()
    return _NC_CACHE


def make_in_maps(x, Wq, Wk, Wv, Wo):
    x = np.asarray(x, dtype=np.float32)
    xT = _round_f32r(x.reshape(BT, C).T)
    cosT, sinT = _rope_tables()
    in_maps = []
    csfull = np.concatenate([cosT, sinT], axis=0)        # [256, BT]
    if STAGE_BF16:
        import ml_dtypes

        def conv(a):
            return np.ascontiguousarray(a).astype(ml_dtypes.bfloat16)
    else:
        def conv(a):
            return _round_f32r(np.ascontiguousarray(a, dtype=np.float32))
    def wlay(wT):
        # [C, CL] -> [128, CT*CL] with partition p holding WT[ct*128+p, :]
        return np.ascontiguousarray(
            wT.reshape(CT, 128, CL).transpose(1, 0, 2).reshape(128, CT * CL))

    for m in range(NCORES):
        sl = slice(m * CL, (m + 1) * CL)
        in_maps.append({
            "xts": conv(xT[sl, :]),
            "wqT": conv(wlay(np.asarray(Wq)[sl, :].T)),
            "wkT": conv(wlay(np.asarray(Wk)[sl, :].T)),
            "wvT": conv(wlay(np.asarray(Wv)[sl, :].T)),
            "woT": conv(wlay(np.asarray(Wo)[sl, :].T)),
            "css": np.ascontiguousarray(csfull[m * CSH:(m + 1) * CSH, :]),
        })
    return in_maps


def kernel(x, Wq, Wk, Wv, Wo, _trace=False):
    in_maps = make_in_maps(x, Wq, Wk, Wv, Wo)
    nc = _get_nc()
    res = run_bass_kernel_spmd(nc, in_maps, list(range(NCORES)),
                               trace=_trace)
    outs = []
    for m in range(NCORES):
        arr = res.results[m]["out"].reshape(128, 4, 4, CL)
        outs.append(arr.transpose(1, 2, 0, 3).reshape(BT, CL))
    out = np.ascontiguousarray(np.concatenate(outs, axis=1))
    out = out.reshape(B, T, C)
    if _trace:
        return out, res
    return out



# revision 4
# speedup vs baseline: 1.0231x; 1.0231x over previous
"""Causal self-attention with RoPE (B=2, T=1024, C=2048, H=16) on 8 TRN2
NeuronCores, head-parallel tensor sharding (2 heads per core).

Kernel I/O staging (ExternalInput/Output DMA) is far slower than HBM, so the
kernel minimizes both staged bytes and staged-DMA descriptor count:
  - ALL per-core inputs ride in ONE flat bf16 blob [1, 2.65M]: the x^T
    channel shard, the four weight shards, and a compact bf16 cos/sin shard
    (T-only; batch duplication is an index trick). Each segment is copied
    DRAM->DRAM (flat, 64KB descriptors) to internal-DRAM mirrors, spread
    across the three DMA queues (SP-HWDGE / Act-HWDGE / Pool-SWDGE); all
    SBUF loads then read fast internal DRAM.
  - Output is staged chunk-major: each 512-token chunk lands contiguously in
    an internal bf16 buffer and is copied flat to the ExternalOutput right
    after it is computed, overlapping the remaining compute.
Compute (same structure as the tuned baseline):
  1. x^T AllGathered on-chip from the per-core channel shards.
  2. QKV projections with bf16 matmuls (contraction dim on partitions).
  3. RoPE via an SBUF partition-shift DMA + DVE FMA.
  4. Causal attention in [tk, tq] score layout: exp on ScalarE from PSUM,
     softmax denominator via an all-ones-lhsT matmul, unnormalized y in
     PSUM, one reciprocal + multiply at the end; fully-masked tiles skipped.
  5. AllGather of the per-core y^T shard (one per batch, overlapped).
  6. Output projection of this core's 256 output columns.
Host reassembles: upcast bf16 -> f32, concat core outputs on channel dim.
"""
import numpy as np

import concourse.bass as bass
import concourse.mybir as mybir
import concourse.tile as tile
from concourse import bacc
from concourse.bass_utils import run_bass_kernel_spmd

F32 = mybir.dt.float32
F32R = mybir.dt.float32r
BF16 = mybir.dt.bfloat16

B, T, C = 2, 1024, 2048
H = 16
D = C // H            # 128
BT = B * T            # 2048
NCORES = 8
HL = H // NCORES      # heads per core = 2
CL = HL * D           # local channels = 256
ATT_SCALE = 1.0 / float(np.sqrt(D))
ROPE_BASE = 10000.0
NEG = -1.0e30

CT = C // 128         # 16 contraction tiles
TB = BT // 512        # 4 token blocks of 512
CSH = 2 * D // NCORES  # rows of the cos/sin shard per core = 32
RG = [list(range(NCORES))]

SEG = CL * BT          # 524288 elements per 1MB bf16 segment
CSSEG = CSH * T        # 32768 elements for the cos/sin shard
BLOB = 5 * SEG + CSSEG  # flat input blob length (elements)
OUTCH = 128 * 1024     # output chunk: 128 partitions x 1024 cols


def _rope_tables():
    inv_freq = 1.0 / (ROPE_BASE ** (np.arange(0, D, 2, dtype=np.float64) / D))
    t = np.arange(T, dtype=np.float64)
    freqs = np.outer(t, inv_freq)                       # [T, D/2]
    emb = np.concatenate([freqs, freqs], axis=-1)        # [T, D]
    cos = np.cos(emb).astype(np.float32)                 # [T, D]
    sin = np.sin(emb).astype(np.float32)
    cosT = np.ascontiguousarray(cos.T)                   # [D, T]
    sinT = np.ascontiguousarray(sin.T)
    sgn_sinT = sinT.copy()
    sgn_sinT[: D // 2] *= -1.0                           # rotate_half sign
    return cosT, sgn_sinT


def _build(use_collective=True, reps=1):
    nc = bacc.Bacc("TRN2", target_bir_lowering=False, debug=False,
                   num_devices=NCORES)

    # one flat staged input blob per core:
    # [xts | wq | wk | wv | wo | css]  (bf16)
    blob_d = nc.dram_tensor("blob", [1, BLOB], BF16, kind="ExternalInput").ap()
    # output, chunk-major: chunk bh holds [128, 1024] contiguously
    out_d = nc.dram_tensor("out", [4, OUTCH], BF16,
                           kind="ExternalOutput").ap()

    # internal DRAM (fast HBM) mirrors of the blob segments
    xtsb_d = nc.dram_tensor("xtsb", [CL, BT], BF16)
    wmir_d = [nc.dram_tensor(f"wmir{i}", [128, CT * CL], BF16)
              for i in range(4)]
    cssb_d = nc.dram_tensor("cssb", [CSH, T], BF16)
    xtg_d = nc.dram_tensor("xtg", [C, BT], BF16, addr_space="Shared")
    csg_d = nc.dram_tensor("csg", [NCORES * CSH, T], BF16,
                           addr_space="Shared")
    yin_d = [nc.dram_tensor(f"yin{b}", [CL, T], F32R) for b in range(B)]
    yout_d = [nc.dram_tensor(f"yout{b}", [C, T], F32R, addr_space="Shared")
              for b in range(B)]
    obuf_d = nc.dram_tensor("obuf", [4, OUTCH], BF16)

    with tile.TileContext(nc) as tc:
        with (
            tc.tile_pool(name="wpool", bufs=1) as wpool,
            tc.tile_pool(name="const", bufs=1) as cpool,
            tc.tile_pool(name="qkv", bufs=1) as qkvpool,
            tc.tile_pool(name="xs", bufs=6) as xspool,
            tc.tile_pool(name="rope", bufs=2) as ropepool,
            tc.tile_pool(name="att", bufs=3) as attpool,
            tc.tile_pool(name="wo", bufs=4) as wopool,
        ):
          for _rep in range(reps):
            # ---- stage blob segments into internal DRAM (flat DRAM->DRAM
            # copies, 64KB descriptors, spread over the three DMA queues) ----
            nc.sync.dma_start(out=xtsb_d.ap(), in_=blob_d[0:1, 0:SEG])
            nc.scalar.dma_start(out=cssb_d.ap(),
                                in_=blob_d[0:1, 5 * SEG:5 * SEG + CSSEG])
            nc.sync.dma_start(out=wmir_d[0].ap(),
                              in_=blob_d[0:1, SEG:2 * SEG])
            nc.scalar.dma_start(out=wmir_d[1].ap(),
                                in_=blob_d[0:1, 2 * SEG:3 * SEG])
            nc.scalar.dma_start(out=wmir_d[2].ap(),
                                in_=blob_d[0:1, 3 * SEG:4 * SEG])
            nc.gpsimd.dma_start(out=wmir_d[3].ap(),
                                in_=blob_d[0:1, 4 * SEG:5 * SEG])
            if use_collective:
                nc.gpsimd.collective_compute(
                    "AllGather", mybir.AluOpType.bypass, replica_groups=RG,
                    ins=[xtsb_d.ap()], outs=[xtg_d.ap()])
                nc.gpsimd.collective_compute(
                    "AllGather", mybir.AluOpType.bypass, replica_groups=RG,
                    ins=[cssb_d.ap()], outs=[csg_d.ap()])
            else:
                nc.gpsimd.dma_start(out=xtg_d.ap()[0:CL, :], in_=xtsb_d.ap())
                nc.gpsimd.dma_start(out=csg_d.ap()[0:CSH, :], in_=cssb_d.ap())

            # ---- weights: SBUF loads from the internal mirrors (fast) ----
            wq_sb = wpool.tile([128, CT * CL], BF16, tag="wq")
            wk_sb = wpool.tile([128, CT * CL], BF16, tag="wk")
            wv_sb = wpool.tile([128, CT * CL], BF16, tag="wv")
            wo_sb = wpool.tile([128, CT * CL], F32R, tag="wo")
            wo_ld = wpool.tile([128, CT * CL], BF16, tag="wold")
            for w_sb, w_d, eng in ((wq_sb, wmir_d[0], nc.sync),
                                   (wk_sb, wmir_d[1], nc.scalar),
                                   (wv_sb, wmir_d[2], nc.scalar),
                                   (wo_ld, wmir_d[3], nc.gpsimd)):
                eng.dma_start(out=w_sb[:], in_=w_d.ap())
            nc.vector.tensor_copy(wo_sb[:], wo_ld[:])

            # gathered table is [cosT(128 rows); sgn_sinT(128 rows)], T cols
            cs_ld = cpool.tile([D, T], BF16, tag="cosld")
            sn_ld = cpool.tile([D, T], BF16, tag="sinld")
            nc.scalar.dma_start(out=cs_ld[:], in_=csg_d.ap()[0:D, :])
            nc.sync.dma_start(out=sn_ld[:], in_=csg_d.ap()[D:2 * D, :])
            cos_sb = cpool.tile([D, T], F32, tag="cos")
            sin_sb = cpool.tile([D, T], F32, tag="sin")
            nc.vector.tensor_copy(cos_sb[:], cs_ld[:])
            nc.vector.tensor_copy(sin_sb[:], sn_ld[:])

            ones_f = cpool.tile([128, 128], F32, tag="onesf")
            nc.gpsimd.memset(ones_f[:], 1.0)
            ones_sb = cpool.tile([128, 128], F32R, tag="ones")
            nc.vector.tensor_copy(ones_sb[:], ones_f[:])

            # additive causal mask for diagonal 128x128 blocks:
            # rows=tk, cols=tq; keep (0.0) where tk <= tq else NEG
            mask_sb = cpool.tile([128, 128], F32, tag="mask")
            nc.gpsimd.memset(mask_sb[:], 0.0)
            nc.gpsimd.affine_select(
                out=mask_sb[:], in_=mask_sb[:],
                compare_op=mybir.AluOpType.is_ge,
                fill=NEG, base=0,
                pattern=[[1, 128]], channel_multiplier=-1,
            )

            # persistent qkv/y activations
            qT = [qkvpool.tile([D, BT], F32R, tag=f"qT{h}", name=f"qT{h}")
                  for h in range(HL)]
            kT = [qkvpool.tile([D, BT], F32R, tag=f"kT{h}", name=f"kT{h}")
                  for h in range(HL)]
            v_sb = qkvpool.tile([128, (BT // 128) * CL], F32R, tag="v")
            yT = [qkvpool.tile([D, BT], F32R, tag=f"yT{h}", name=f"yT{h}")
                  for h in range(HL)]

            # ---- phase 1: QKV projections + rope ----
            with tc.tile_pool(name="psqkv", bufs=1, space="PSUM") as psq:
                for tb in range(TB):
                    tcol = tb * 512
                    ccol = tcol - (tb // 2) * T   # col into T-wide tables
                    ps_q = [psq.tile([128, 512], F32, tag=f"pq{h}", name=f"pq{h}")
                            for h in range(HL)]
                    ps_k = [psq.tile([128, 512], F32, tag=f"pk{h}", name=f"pk{h}")
                            for h in range(HL)]
                    ps_v = [psq.tile([128, CL], F32, tag=f"pv{i}", name=f"pv{i}")
                            for i in range(4)]
                    for ct in range(CT):
                        xs = xspool.tile([128, 512], BF16, tag="xs")
                        eng = nc.sync if ct % 2 == 0 else nc.scalar
                        eng.dma_start(
                            out=xs[:],
                            in_=xtg_d.ap()[ct * 128:(ct + 1) * 128,
                                           tcol:tcol + 512],
                        )
                        st, sp = ct == 0, ct == CT - 1
                        for h in range(HL):
                            nc.tensor.matmul(
                                ps_q[h][:],
                                wq_sb[:, ct * CL + h * D: ct * CL + (h + 1) * D],
                                xs[:], start=st, stop=sp)
                            nc.tensor.matmul(
                                ps_k[h][:],
                                wk_sb[:, ct * CL + h * D: ct * CL + (h + 1) * D],
                                xs[:], start=st, stop=sp)
                        for i in range(4):
                            nc.tensor.matmul(
                                ps_v[i][:],
                                xs[:, i * 128:(i + 1) * 128],
                                wv_sb[:, ct * CL:(ct + 1) * CL],
                                start=st, stop=sp)
                    # rope on q, k; plain copy for v
                    for h in range(HL):
                        for name, ps, dst in (("q", ps_q[h], qT[h]),
                                              ("k", ps_k[h], kT[h])):
                            tmp = ropepool.tile([128, 512], F32, tag="rtmp")
                            nc.vector.tensor_copy(tmp[:], ps[:])
                            rot = ropepool.tile([128, 512], F32, tag="rrot")
                            nc.gpsimd.dma_start(out=rot[0:64, :],
                                                in_=tmp[64:128, :])
                            nc.gpsimd.dma_start(out=rot[64:128, :],
                                                in_=tmp[0:64, :])
                            t1 = ropepool.tile([128, 512], F32, tag="rt1")
                            nc.vector.tensor_mul(
                                t1[:], ps[:], cos_sb[:, ccol:ccol + 512])
                            t2 = ropepool.tile([128, 512], F32, tag="rt2")
                            nc.vector.tensor_mul(
                                t2[:], rot[:], sin_sb[:, ccol:ccol + 512])
                            nc.vector.tensor_add(
                                dst[:, tcol:tcol + 512], t1[:], t2[:])
                    for i in range(4):
                        gt = tb * 4 + i
                        nc.vector.tensor_copy(
                            v_sb[:, gt * CL:(gt + 1) * CL], ps_v[i][:])

            # ---- phase 2: attention + AllGather per batch ----
            with tc.tile_pool(name="psatt", bufs=1, space="PSUM") as psa:
                for b in range(B):
                    bcol = b * T
                    for h in range(HL):
                        for jj in range(2):
                            qcol = bcol + jj * 512
                            njt = 4 * jj + 4
                            ps_y = psa.tile([128, 512], F32, tag="y",
                                            bufs=2)
                            ps_l = psa.tile([128, 512], F32, tag="l",
                                            bufs=2)
                            for j in range(njt):
                                c0 = max(0, j * 128 - jj * 512)
                                ps_s = psa.tile([128, 512], F32, tag="s",
                                                bufs=3)
                                nc.tensor.matmul(
                                    ps_s[:, c0:512],
                                    kT[h][:, bcol + j * 128: bcol + (j + 1) * 128],
                                    qT[h][:, qcol + c0: qcol + 512],
                                    start=True, stop=True)
                                diag0 = j * 128 - jj * 512
                                if 0 <= diag0 < 512:
                                    nc.vector.tensor_add(
                                        ps_s[:, diag0:diag0 + 128],
                                        ps_s[:, diag0:diag0 + 128],
                                        mask_sb[:])
                                p = attpool.tile([128, 512], F32R, tag="p")
                                nc.scalar.activation(
                                    p[:, c0:512], ps_s[:, c0:512],
                                    mybir.ActivationFunctionType.Exp,
                                    scale=ATT_SCALE)
                                st, sp = j == 0, j == njt - 1
                                nc.tensor.matmul(
                                    ps_l[:, c0:512], ones_sb[:],
                                    p[:, c0:512], start=st, stop=sp)
                                gt = (bcol // 128) + j
                                nc.tensor.matmul(
                                    ps_y[:, c0:512],
                                    v_sb[:, gt * CL + h * D: gt * CL + (h + 1) * D],
                                    p[:, c0:512], start=st, stop=sp)
                            rec = attpool.tile([128, 512], F32, tag="rec")
                            nc.vector.reciprocal(rec[:], ps_l[:])
                            nc.vector.tensor_mul(
                                yT[h][:, qcol:qcol + 512], ps_y[:], rec[:])
                    # ship this batch's yT shard and AllGather it
                    for h in range(HL):
                        nc.sync.dma_start(
                            out=yin_d[b].ap()[h * D:(h + 1) * D, :],
                            in_=yT[h][:, bcol:bcol + T])
                    if use_collective:
                        nc.gpsimd.collective_compute(
                            "AllGather", mybir.AluOpType.bypass,
                            replica_groups=RG,
                            ins=[yin_d[b].ap()],
                            outs=[yout_d[b].ap()],
                        )
                    else:
                        nc.gpsimd.dma_start(out=yout_d[b].ap()[0:CL, :],
                                            in_=yin_d[b].ap())

            # ---- phase 3: output projection (this core's 256 columns) ----
            # each bh chunk: PSUM -> bf16 SBUF -> internal obuf (fast) ->
            # flat copy to the staged output, overlapping later chunks
            ob_big = qkvpool.tile([128, CT * CL], BF16, tag="obig")
            with tc.tile_pool(name="pso", bufs=1, space="PSUM") as pso:
                for b in range(B):
                    for half in range(2):
                        bh = b * 2 + half
                        hcol = half * 512
                        ps_o = [pso.tile([128, CL], F32, tag=f"po{i}", name=f"po{i}")
                                for i in range(4)]
                        for ct in range(CT):
                            yg = wopool.tile([128, 512], F32R, tag="yg")
                            eng = nc.sync if ct % 2 == 0 else nc.scalar
                            eng.dma_start(
                                out=yg[:],
                                in_=yout_d[b].ap()[ct * 128:(ct + 1) * 128,
                                                   hcol:hcol + 512])
                            st, sp = ct == 0, ct == CT - 1
                            for i in range(4):
                                nc.tensor.matmul(
                                    ps_o[i][:],
                                    yg[:, i * 128:(i + 1) * 128],
                                    wo_sb[:, ct * CL:(ct + 1) * CL],
                                    start=st, stop=sp)
                        for i in range(4):
                            nc.vector.tensor_copy(
                                ob_big[:, bh * 1024 + i * CL:
                                       bh * 1024 + (i + 1) * CL],
                                ps_o[i][:])
                        nc.gpsimd.dma_start(
                            out=obuf_d.ap()[bh:bh + 1, :],
                            in_=ob_big[:, bh * 1024:(bh + 1) * 1024])
                        eng = nc.sync if bh % 2 == 0 else nc.scalar
                        eng.dma_start(
                            out=out_d[bh:bh + 1, :],
                            in_=obuf_d.ap()[bh:bh + 1, :])

    nc.compile()
    return nc


_NC_CACHE = None


def _get_nc():
    global _NC_CACHE
    if _NC_CACHE is None:
        _NC_CACHE = _build()
    return _NC_CACHE


def make_in_maps(x, Wq, Wk, Wv, Wo):
    import ml_dtypes

    x = np.asarray(x, dtype=np.float32)
    xT = np.ascontiguousarray(x.reshape(BT, C).T)        # [C, BT]
    cosT, sinT = _rope_tables()
    csfull = np.concatenate([cosT, sinT], axis=0)        # [256, T]

    def conv(a):
        return np.ascontiguousarray(a).astype(ml_dtypes.bfloat16)

    def wlay(wT):
        # [C, CL] -> [128, CT*CL] with partition p holding WT[ct*128+p, :]
        return np.ascontiguousarray(
            wT.reshape(CT, 128, CL).transpose(1, 0, 2).reshape(128, CT * CL))

    in_maps = []
    for m in range(NCORES):
        sl = slice(m * CL, (m + 1) * CL)
        segs = [
            conv(xT[sl, :]).reshape(-1),
            conv(wlay(np.asarray(Wq)[sl, :].T)).reshape(-1),
            conv(wlay(np.asarray(Wk)[sl, :].T)).reshape(-1),
            conv(wlay(np.asarray(Wv)[sl, :].T)).reshape(-1),
            conv(wlay(np.asarray(Wo)[sl, :].T)).reshape(-1),
            conv(csfull[m * CSH:(m + 1) * CSH, :]).reshape(-1),
        ]
        in_maps.append({"blob": np.concatenate(segs).reshape(1, BLOB)})
    return in_maps


def kernel(x, Wq, Wk, Wv, Wo, _trace=False):
    in_maps = make_in_maps(x, Wq, Wk, Wv, Wo)
    nc = _get_nc()
    res = run_bass_kernel_spmd(nc, in_maps, list(range(NCORES)),
                               trace=_trace)
    outs = []
    for m in range(NCORES):
        arr = np.asarray(res.results[m]["out"]).astype(np.float32)
        arr = arr.reshape(4, 128, 4, CL)                 # (bh, p, i, c)
        outs.append(arr.transpose(0, 2, 1, 3).reshape(BT, CL))
    out = np.ascontiguousarray(np.concatenate(outs, axis=1))
    out = out.reshape(B, T, C)
    if _trace:
        return out, res
    return out
